# revision 2
# baseline (speedup 1.0000x reference)
"""Trainium2 Bass kernel for Custom_RoPE (rotate-half RoPE + per-(head,token)
min/max observer).

Reference computation (float branch):
    out = x * cos + rotate_half(x) * sin        # (H, T, D)
    obs_max = max(out, axis=-1)                 # (H, T)
    obs_min = min(out, axis=-1)

Sharding: tokens (T) are split across the 8 NeuronCores (1024 tokens each).
All math is independent per (head, token), so no communication is needed, and
T-sharding also shards the cos/sin reads (vs. duplicating them 8x under
head-sharding).

Per-core layout: SBUF tiles hold 128 tokens on the partition axis and
(head, d) on the free axis.  rotate_half is expressed as a reversed
access-pattern read of the x tile; the sign flip is pre-baked into an
"sm" tile (-sin for d<64, +sin for d>=64) built once on ScalarE, so

    out = x * cos + swap(x) * sm

is two tensor_tensor mults and one add per tile.  The elementwise work is
split between VectorE and GpSimd by head range (they use disjoint SBUF ports
for 1x tensor_tensor ops, so they run concurrently); the free-axis min/max
reductions can only run on VectorE.  All DMA goes through HWDGE (nc.sync) so
the Q7 cores are never needed for descriptor generation.
"""

import numpy as np

import concourse.bacc as bacc
import concourse.mybir as mybir
from concourse import bass_utils
from concourse.tile import TileContext

H, T, D = 32, 8192, 128
NCORES = 8
TL = T // NCORES  # tokens per core
P = 128  # partitions = tokens per block
NBLK = TL // P  # token blocks per core
HALF = D // 2
F32 = mybir.dt.float32

# Heads per block whose mult/mult/add run on GpSimd (rest on VectorE).
# GpSimd 2-input ops are ~2.2x slower per element than DVE, but run in
# parallel; DVE additionally owns the two reductions.
GP_HEADS = 14

# Fuse the add with the max-reduction via tensor_tensor_reduce (per-head
# instructions, accum_out is [P,1]) instead of a batched add + batched
# reduce_max.  NOTE: the TENSOR_TENSOR_REDUCE ISA opcode fails at runtime on
# this deployment (works in CoreSim), so this must stay False.
USE_TTR = False

_CACHE = {}


def _build(gp_heads=GP_HEADS, use_ttr=USE_TTR):
    nc = bacc.Bacc("TRN2", target_bir_lowering=False, debug=False, num_devices=NCORES)
    x = nc.dram_tensor("x", (H, TL, D), F32, kind="ExternalInput")
    cos = nc.dram_tensor("cos", (TL, D), F32, kind="ExternalInput")
    sin = nc.dram_tensor("sin", (TL, D), F32, kind="ExternalInput")
    out = nc.dram_tensor("out", (H, TL, D), F32, kind="ExternalOutput")
    # Token-major observer outputs; host transposes to (H, TL).  This keeps
    # the DMA store's innermost dim contiguous in DRAM.
    omax = nc.dram_tensor("omax", (TL, H), F32, kind="ExternalOutput")
    omin = nc.dram_tensor("omin", (TL, H), F32, kind="ExternalOutput")

    mult = mybir.AluOpType.mult
    add = mybir.AluOpType.add

    with TileContext(nc) as tc:
        with (
            tc.tile_pool(name="const", bufs=1) as constp,
            tc.tile_pool(name="io", bufs=2) as io,
            tc.tile_pool(name="tmp", bufs=2) as tmp,
        ):
            # --- whole-core constant loads: cos/sin for all 8 blocks ---
            ct_all = constp.tile([P, NBLK, D], F32)
            st_all = constp.tile([P, NBLK, D], F32)
            nc.sync.dma_start(
                ct_all[:, :, :], cos.ap().rearrange("(b p) d -> p b d", p=P)
            )
            nc.sync.dma_start(
                st_all[:, :, :], sin.ap().rearrange("(b p) d -> p b d", p=P)
            )
            # sm = concat(-sin_half1, sin_half2) along d, for every block
            sm_all = constp.tile([P, NBLK, D], F32)
            nc.scalar.mul(sm_all[:, :, 0:HALF], st_all[:, :, 0:HALF], -1.0)
            nc.scalar.copy(sm_all[:, :, HALF:D], st_all[:, :, HALF:D])

            # persistent observer accumulators
            omax_t = constp.tile([P, NBLK, H], F32)
            omin_t = constp.tile([P, NBLK, H], F32)

            for b in range(NBLK):
                bs = slice(b * P, (b + 1) * P)
                xt = io.tile([P, H, D], F32, tag="x")
                nc.sync.dma_start(
                    xt[:, :, :], x.ap()[:, bs, :].rearrange("h p d -> p h d")
                )
                t1 = tmp.tile([P, H, D], F32, tag="t1")
                t2 = tmp.tile([P, H, D], F32, tag="t2")
                ot = io.tile([P, H, D], F32, tag="o")

                ct = ct_all[:, b, :]
                sm = sm_all[:, b, :]
                sm4 = sm.rearrange("p (two half) -> p two half", two=2)
                xt4 = xt.rearrange("p h (two half) -> p h two half", two=2)
                t24 = t2.rearrange("p h (two half) -> p h two half", two=2)

                # head-range split across engines
                splits = []
                if gp_heads > 0:
                    splits.append((nc.gpsimd, 0, gp_heads))
                if gp_heads < H:
                    splits.append((nc.vector, gp_heads, H))

                for eng, h0, h1 in splits:
                    hg = h1 - h0
                    # t1 = x * cos
                    eng.tensor_tensor(
                        t1[:, h0:h1, :],
                        xt[:, h0:h1, :],
                        ct[:, None, :].broadcast_to((P, hg, D)),
                        mult,
                    )
                    # t2 = swap(x) * sm
                    eng.tensor_tensor(
                        t24[:, h0:h1, :, :],
                        xt4[:, h0:h1, ::-1, :],
                        sm4[:, None, :, :].broadcast_to((P, hg, 2, HALF)),
                        mult,
                    )

                if use_ttr:
                    # GpSimd heads: plain add (max handled by batched reduce
                    # below would double-read; instead run per-head TTR on
                    # DVE for DVE heads and batched reduce for gp heads).
                    if gp_heads > 0:
                        nc.gpsimd.tensor_tensor(
                            ot[:, 0:gp_heads, :],
                            t1[:, 0:gp_heads, :],
                            t2[:, 0:gp_heads, :],
                            add,
                        )
                        nc.vector.tensor_reduce(
                            omax_t[:, b, 0:gp_heads],
                            ot[:, 0:gp_heads, :],
                            axis=mybir.AxisListType.X,
                            op=mybir.AluOpType.max,
                        )
                    for h in range(gp_heads, H):
                        nc.vector.tensor_tensor_reduce(
                            out=ot[:, h, :],
                            in0=t1[:, h, :],
                            in1=t2[:, h, :],
                            scale=1.0,
                            scalar=float(np.finfo(np.float32).min),
                            op0=add,
                            op1=mybir.AluOpType.max,
                            accum_out=omax_t[:, b, h : h + 1],
                        )
                else:
                    for eng, h0, h1 in splits:
                        eng.tensor_tensor(
                            ot[:, h0:h1, :], t1[:, h0:h1, :], t2[:, h0:h1, :], add
                        )
                    nc.vector.tensor_reduce(
                        omax_t[:, b, :],
                        ot[:, :, :],
                        axis=mybir.AxisListType.X,
                        op=mybir.AluOpType.max,
                    )

                nc.vector.tensor_reduce(
                    omin_t[:, b, :],
                    ot[:, :, :],
                    axis=mybir.AxisListType.X,
                    op=mybir.AluOpType.min,
                )

                nc.sync.dma_start(
                    out.ap()[:, bs, :].rearrange("h p d -> p h d"), ot[:, :, :]
                )

            nc.sync.dma_start(
                omax.ap().rearrange("(b p) h -> p b h", p=P), omax_t[:, :, :]
            )
            nc.sync.dma_start(
                omin.ap().rearrange("(b p) h -> p b h", p=P), omin_t[:, :, :]
            )

    nc.compile()
    return nc


def get_nc(gp_heads=GP_HEADS, use_ttr=USE_TTR):
    key = (gp_heads, use_ttr)
    if key not in _CACHE:
        _CACHE[key] = _build(gp_heads, use_ttr)
    return _CACHE[key]


def kernel(x, scale_x, cos, scale_cos, sin, scale_sin, **run_kwargs):
    x = np.asarray(x, dtype=np.float32)
    cos = np.asarray(cos, dtype=np.float32)
    sin = np.asarray(sin, dtype=np.float32)
    assert x.shape == (H, T, D), x.shape

    nc = get_nc()
    in_maps = []
    for c in range(NCORES):
        ts = slice(c * TL, (c + 1) * TL)
        in_maps.append(
            {
                "x": np.ascontiguousarray(x[:, ts, :]),
                "cos": np.ascontiguousarray(cos[ts, :]),
                "sin": np.ascontiguousarray(sin[ts, :]),
            }
        )
    res = bass_utils.run_bass_kernel_spmd(
        nc, in_maps, core_ids=list(range(NCORES)), **run_kwargs
    )
    out = np.concatenate([res.results[c]["out"] for c in range(NCORES)], axis=1)
    obs_max = np.concatenate(
        [res.results[c]["omax"].T for c in range(NCORES)], axis=1
    )
    obs_min = np.concatenate(
        [res.results[c]["omin"].T for c in range(NCORES)], axis=1
    )
    kernel.last_results = res
    return out, obs_max, obs_min


# revision 3
# speedup vs baseline: 1.3457x; 1.3457x over previous
"""Trainium2 Bass kernel for Custom_RoPE (rotate-half RoPE + per-(head,token)
min/max observer).

Reference computation (float branch):
    out = x * cos + rotate_half(x) * sin        # (H, T, D)
    obs_max = max(out, axis=-1)                 # (H, T)
    obs_min = min(out, axis=-1)

Sharding: tokens (T) are split across the 8 NeuronCores (1024 tokens each).
All math is independent per (head, token), so no communication is needed, and
T-sharding also shards the cos/sin reads (vs. duplicating them 8x under
head-sharding).

Per-core dataflow (mode="pe", the fast path).  SBUF tiles hold KT=4
consecutive tokens per partition (so HBM DMA descriptors are KT*D*4 = 2 KiB,
not 512 B) and (head, token-in-partition, d) on the free axis.  The sign flip
of rotate-half is pre-baked into an "sm" tile (-sin for d<64, +sin for d>=64)
built once on ScalarE, so   out = x*cos + swap(x)*sm.

Engine assignment per wave of WH=4 heads (hardware-measured constraint: DVE
2-source ops and GpSimd fully serialize on the shared SBUF port, while DVE
1-source ops (tensor_reduce) run concurrently with GpSimd at full speed):

    VectorE : t1 = x * cos      (the only 2-src DVE op)
              reduce_max / reduce_min  (1-src; overlap GpSimd work)
    GpSimd  : t2 = swap(x) * sm (two half-D tensor_tensor mults)
    TensorE : psum = I @ t1 + I @ t2   (identity matmuls, fp32-exact add)
    ScalarE : ot = copy(psum)   (PSUM -> SBUF; DMA cannot read PSUM)
    Sync    : all DMA (HWDGE; never touches the Q7 cores)

mode="dve" keeps the add on VectorE (no PE/PSUM involvement) as a fallback.
"""

import numpy as np

import concourse.bacc as bacc
import concourse.mybir as mybir
from concourse import bass_utils
from concourse.masks import make_identity
from concourse.tile import TileContext

H, T, D = 32, 8192, 128
NCORES = 8
TL = T // NCORES  # tokens per core
P = 128  # SBUF partitions
KT = 4  # consecutive tokens per partition
TB = P * KT  # tokens per block (512)
NBLK = TL // TB  # blocks per core (2)
WH = 4  # heads per wave
NWAVE = H // WH  # waves per block (8)
HALF = D // 2
F32 = mybir.dt.float32

MODE = "pe"  # "pe" | "dve"

_CACHE = {}


def _build(mode=MODE):
    nc = bacc.Bacc("TRN2", target_bir_lowering=False, debug=False, num_devices=NCORES)
    x = nc.dram_tensor("x", (H, TL, D), F32, kind="ExternalInput")
    cos = nc.dram_tensor("cos", (TL, D), F32, kind="ExternalInput")
    sin = nc.dram_tensor("sin", (TL, D), F32, kind="ExternalInput")
    out = nc.dram_tensor("out", (H, TL, D), F32, kind="ExternalOutput")
    # Token-major observer outputs; host transposes to (H, TL).  Keeps the
    # DMA store's innermost dim contiguous in DRAM.
    omax = nc.dram_tensor("omax", (TL, H), F32, kind="ExternalOutput")
    omin = nc.dram_tensor("omin", (TL, H), F32, kind="ExternalOutput")

    mult = mybir.AluOpType.mult
    add = mybir.AluOpType.add

    with TileContext(nc) as tc:
        with (
            tc.tile_pool(name="const", bufs=1) as constp,
            tc.tile_pool(name="io", bufs=3) as io,
            tc.tile_pool(name="tmp", bufs=3) as tmp,
            tc.tile_pool(name="psum", bufs=2, space="PSUM") as psump,
        ):
            ident = constp.tile([P, P], F32)
            make_identity(nc, ident)

            # cos / sm for the whole core: (P, NBLK, KT, D)
            ct_all = constp.tile([P, NBLK, KT, D], F32)
            st_all = constp.tile([P, NBLK, KT, D], F32)
            sm_all = constp.tile([P, NBLK, KT, D], F32)
            for b in range(NBLK):
                bs = slice(b * TB, (b + 1) * TB)
                nc.sync.dma_start(
                    ct_all[:, b].rearrange("p k d -> p (k d)"),
                    cos.ap()[bs, :].rearrange("(p k) d -> p (k d)", p=P),
                )
                nc.sync.dma_start(
                    st_all[:, b].rearrange("p k d -> p (k d)"),
                    sin.ap()[bs, :].rearrange("(p k) d -> p (k d)", p=P),
                )
            nc.scalar.mul(sm_all[:, :, :, 0:HALF], st_all[:, :, :, 0:HALF], -1.0)
            nc.scalar.copy(sm_all[:, :, :, HALF:D], st_all[:, :, :, HALF:D])

            # persistent observer accumulators: (P, NBLK, KT, H), h contiguous
            omax_t = constp.tile([P, NBLK, KT, H], F32)
            omin_t = constp.tile([P, NBLK, KT, H], F32)

            for b in range(NBLK):
                bs = slice(b * TB, (b + 1) * TB)
                cb = ct_all[:, b].rearrange("p k d -> p (k d)")  # (P, KT*D)
                smb = sm_all[:, b]  # (P, KT, D)
                for w in range(NWAVE):
                    h0, h1 = w * WH, (w + 1) * WH
                    xw = io.tile([P, WH, KT * D], F32, tag="x")
                    nc.sync.dma_start(
                        xw[:, :, :],
                        x.ap()[h0:h1, bs, :].rearrange("h (p k) d -> p h (k d)", p=P),
                    )
                    xw4 = xw.rearrange("p h (k d) -> p h k d", d=D)

                    # t1 = x * cos  (VectorE, 2-src)
                    t1 = tmp.tile([P, WH, KT * D], F32, tag="t1")
                    nc.vector.tensor_tensor(
                        t1[:, :, :],
                        xw[:, :, :],
                        cb[:, None, :].broadcast_to((P, WH, KT * D)),
                        mult,
                    )

                    # t2 = swap(x) * sm  (GpSimd, two half-D mults)
                    t2 = tmp.tile([P, WH, KT, D], F32, tag="t2")
                    nc.gpsimd.tensor_tensor(
                        t2[:, :, :, 0:HALF],
                        xw4[:, :, :, HALF:D],
                        smb[:, None, :, 0:HALF].broadcast_to((P, WH, KT, HALF)),
                        mult,
                    )
                    nc.gpsimd.tensor_tensor(
                        t2[:, :, :, HALF:D],
                        xw4[:, :, :, 0:HALF],
                        smb[:, None, :, HALF:D].broadcast_to((P, WH, KT, HALF)),
                        mult,
                    )
                    t2f = t2.rearrange("p h k d -> p h (k d)")

                    ot = io.tile([P, WH, KT * D], F32, tag="o")
                    if mode == "pe":
                        # add on TensorE: psum = I@t1 + I@t2 (one bank per head)
                        ps = psump.tile([P, WH, KT * D], F32, tag="ps")
                        for i in range(WH):
                            nc.tensor.matmul(
                                ps[:, i, :], ident[:, :], t1[:, i, :],
                                start=True, stop=False,
                            )
                            nc.tensor.matmul(
                                ps[:, i, :], ident[:, :], t2f[:, i, :],
                                start=False, stop=True,
                            )
                        nc.scalar.copy(ot[:, :, :], ps[:, :, :])
                    else:
                        nc.vector.tensor_tensor(
                            ot[:, :, :], t1[:, :, :], t2f[:, :, :], add
                        )

                    # observer reductions (VectorE, 1-src: overlap GpSimd)
                    otr = ot.rearrange("p h (k d) -> p (h k) d", d=D)
                    mx = omax_t[:, b, :, h0:h1].rearrange("p k h -> p h k")
                    mn = omin_t[:, b, :, h0:h1].rearrange("p k h -> p h k")
                    nc.vector.tensor_reduce(
                        mx, otr, axis=mybir.AxisListType.X, op=mybir.AluOpType.max
                    )
                    nc.vector.tensor_reduce(
                        mn, otr, axis=mybir.AxisListType.X, op=mybir.AluOpType.min
                    )

                    nc.sync.dma_start(
                        out.ap()[h0:h1, bs, :].rearrange("h (p k) d -> p h (k d)", p=P),
                        ot[:, :, :],
                    )

            for b in range(NBLK):
                bs = slice(b * TB, (b + 1) * TB)
                nc.sync.dma_start(
                    omax.ap()[bs, :].rearrange("(p k) h -> p k h", p=P),
                    omax_t[:, b, :, :],
                )
                nc.sync.dma_start(
                    omin.ap()[bs, :].rearrange("(p k) h -> p k h", p=P),
                    omin_t[:, b, :, :],
                )

    nc.compile()
    return nc


def get_nc(mode=MODE):
    if mode not in _CACHE:
        _CACHE[mode] = _build(mode)
    return _CACHE[mode]


def kernel(x, scale_x, cos, scale_cos, sin, scale_sin, **run_kwargs):
    x = np.asarray(x, dtype=np.float32)
    cos = np.asarray(cos, dtype=np.float32)
    sin = np.asarray(sin, dtype=np.float32)
    assert x.shape == (H, T, D), x.shape

    nc = get_nc()
    in_maps = []
    for c in range(NCORES):
        ts = slice(c * TL, (c + 1) * TL)
        in_maps.append(
            {
                "x": np.ascontiguousarray(x[:, ts, :]),
                "cos": np.ascontiguousarray(cos[ts, :]),
                "sin": np.ascontiguousarray(sin[ts, :]),
            }
        )
    res = bass_utils.run_bass_kernel_spmd(
        nc, in_maps, core_ids=list(range(NCORES)), **run_kwargs
    )
    out = np.concatenate([res.results[c]["out"] for c in range(NCORES)], axis=1)
    obs_max = np.concatenate(
        [res.results[c]["omax"].T for c in range(NCORES)], axis=1
    )
    obs_min = np.concatenate(
        [res.results[c]["omin"].T for c in range(NCORES)], axis=1
    )
    kernel.last_results = res
    return out, obs_max, obs_min
